# revision 6
# baseline (speedup 1.0000x reference)
"""TRN2 Bass kernel for nn_Attention_3728031613427.

GQA causal attention (B=1, S=2048, D=4096, H=32, KV=8, HD=128) with RoPE,
8-way tensor-parallel over KV heads. Each core computes 4 q-heads / 1 kv-head
and a full [D, S] partial of the output projection; partials are summed on
the host (the all-reduce of the sharding hint).

Host-side prep (outside the measured HW kernel):
  - slice + transpose + bf16-cast weights per core (wqT, wkvT, woT)
  - transpose + cast x -> xT bf16 [D, S]
  - de-interleave the RoPE pair layout via a row permutation of wq/wk so the
    on-device rotation uses contiguous free-dim halves (scores are invariant
    to a permutation applied to both q and k head dims)
  - fold the 1/sqrt(HD) score scale into q's cos/sin tables
  - classify [128 x 512] mask blocks into skip / plain / masked so the device
    loop exploits causal sparsity without hardcoding causality
"""

import sys

if "/opt/trn_rl_repo" not in sys.path:
    sys.path.insert(0, "/opt/trn_rl_repo")

from contextlib import ExitStack

import ml_dtypes
import numpy as np

import concourse.bass as bass
import concourse.mybir as mybir
import concourse.tile as tile
from concourse import bacc
from concourse.bass_utils import run_bass_kernel_spmd

BF16 = mybir.dt.bfloat16
F32 = mybir.dt.float32
NPBF16 = ml_dtypes.bfloat16

B, S, D = 1, 2048, 4096
H, KV, HD = 32, 8, 128
REP = H // KV            # q heads per kv head = heads per core
NCORES = 8
QH = H // NCORES         # 4 q heads per core
SB = 128                 # s-block (query rows per score tile)
TC = 512                 # t-chunk (key cols per score tile)
NSB = S // SB            # 16
NTC = S // TC            # 4
NTB = S // 128           # 16 t-blocks (AV granularity)
DC = D // 128            # 32 contraction chunks for projections
SCHUNK = 512             # s streaming chunk for projections
NSC = S // SCHUNK        # 4
NEG_THRESH = -1e8

# knobs for experiments
_TRACE = False
_LAST_RESULTS = None


def _classify_mask(mask):
    """Per (s-block, t-chunk): 'skip' (all very-negative), 'plain' (all 0),
    or 'masked'. Also per (s-block, t-block 128) inclusion for AV."""
    chunk_kind = []
    av_tblocks = []
    mask_blocks = []
    mask_index = {}
    for i in range(NSB):
        row = []
        for j in range(NTC):
            blk = mask[i * SB:(i + 1) * SB, j * TC:(j + 1) * TC]
            if (blk <= NEG_THRESH).all():
                row.append(("skip", -1))
            elif (blk == 0.0).all():
                row.append(("plain", -1))
            else:
                idx = mask_index.setdefault((i, j), len(mask_blocks))
                if idx == len(mask_blocks):
                    mask_blocks.append(blk)
                row.append(("masked", idx))
        chunk_kind.append(row)
        tbs = []
        for tb in range(NTB):
            sub = mask[i * SB:(i + 1) * SB, tb * 128:(tb + 1) * 128]
            if not (sub <= NEG_THRESH).all() and row[tb * 128 // TC][0] != "skip":
                tbs.append(tb)
        av_tblocks.append(tbs)
    if not mask_blocks:
        mask_blocks.append(np.zeros((SB, TC), np.float32))
    return chunk_kind, av_tblocks, np.stack(mask_blocks).astype(np.float32)


def _build_nc(chunk_kind, av_tblocks, nmask):
    nc = bacc.Bacc()

    xT = nc.declare_dram_parameter("xT", [D, S], BF16, isOutput=False)
    wqT = nc.declare_dram_parameter("wqT", [D, QH * HD], BF16, isOutput=False)
    wkvT = nc.declare_dram_parameter("wkvT", [D, 2 * HD], BF16, isOutput=False)
    woT = nc.declare_dram_parameter("woT", [QH * HD, D], BF16, isOutput=False)
    cosq = nc.declare_dram_parameter("cosq", [S, HD // 2], F32, isOutput=False)
    sinq = nc.declare_dram_parameter("sinq", [S, HD // 2], F32, isOutput=False)
    cosk = nc.declare_dram_parameter("cosk", [S, HD // 2], F32, isOutput=False)
    sink = nc.declare_dram_parameter("sink", [S, HD // 2], F32, isOutput=False)
    maskb = nc.declare_dram_parameter("maskb", [nmask, SB, TC], F32, isOutput=False)
    outT = nc.declare_dram_parameter("outT", [D, S], F32, isOutput=True)

    with tile.TileContext(nc) as tc, ExitStack() as ctx:
        wpool = ctx.enter_context(tc.tile_pool(name="wpool", bufs=1))
        xpool = ctx.enter_context(tc.tile_pool(name="xpool", bufs=2))
        rpool = ctx.enter_context(tc.tile_pool(name="rpool", bufs=1))
        tpool = ctx.enter_context(tc.tile_pool(name="tpool", bufs=8))
        ppool = ctx.enter_context(tc.tile_pool(name="ppool", bufs=2))
        opool = ctx.enter_context(tc.tile_pool(name="opool", bufs=3))
        psum = ctx.enter_context(tc.tile_pool(name="psum", bufs=2, space="PSUM"))

        # --- resident loads ---------------------------------------------
        wq_sb = wpool.tile([128, DC, QH * HD], BF16)
        nc.sync.dma_start(wq_sb, wqT.rearrange("(dc p) m -> p dc m", p=128))
        wkv_sb = wpool.tile([128, DC, 2 * HD], BF16)
        nc.sync.dma_start(wkv_sb, wkvT.rearrange("(dc p) m -> p dc m", p=128))

        cq_sb = wpool.tile([128, NSB, HD // 2], F32)
        nc.sync.dma_start(cq_sb, cosq.rearrange("(i p) f -> p i f", p=128))
        sq_sb = wpool.tile([128, NSB, HD // 2], F32)
        nc.sync.dma_start(sq_sb, sinq.rearrange("(i p) f -> p i f", p=128))
        ck_sb = wpool.tile([128, NSB, HD // 2], F32)
        nc.sync.dma_start(ck_sb, cosk.rearrange("(i p) f -> p i f", p=128))
        sk_sb = wpool.tile([128, NSB, HD // 2], F32)
        nc.sync.dma_start(sk_sb, sink.rearrange("(i p) f -> p i f", p=128))

        qrotT = rpool.tile([128, QH, S], BF16)   # [hd', h, s]
        krotT = rpool.tile([128, S], BF16)       # [hd', t]
        v_sb = rpool.tile([128, NTB, HD], BF16)  # [t-in-block, tb, d']
        attnT = rpool.tile([128, QH, S], BF16)   # [d', h, s]

        # --- phase 1: projections + RoPE --------------------------------
        for sc in range(NSC):
            xc = xpool.tile([128, DC, SCHUNK], BF16, tag="xc")
            nc.sync.dma_start(
                xc, xT[:, sc * SCHUNK:(sc + 1) * SCHUNK].rearrange(
                    "(dc p) s -> p dc s", p=128))
            for ib in range(SCHUNK // SB):
                i = sc * (SCHUNK // SB) + ib
                sblk = slice(ib * SB, (ib + 1) * SB)
                ps_q = psum.tile([128, QH * HD], F32, tag="q")
                ps_kv = psum.tile([128, 2 * HD], F32, tag="kv")
                for dc in range(DC):
                    nc.tensor.matmul(ps_q, xc[:, dc, sblk], wq_sb[:, dc, :],
                                     start=(dc == 0), stop=(dc == DC - 1))
                for dc in range(DC):
                    nc.tensor.matmul(ps_kv, xc[:, dc, sblk], wkv_sb[:, dc, :],
                                     start=(dc == 0), stop=(dc == DC - 1))
                # v: plain copy+cast (normal [t, d'] layout, exactly AV's rhs)
                nc.vector.tensor_copy(v_sb[:, i, :], ps_kv[:, HD:2 * HD])
                # RoPE (de-interleaved pair layout: halves of each head)
                qrot_n = tpool.tile([128, QH * HD], BF16, tag="qrot", bufs=2)
                for h in range(QH):
                    e = slice(h * HD, h * HD + HD // 2)
                    o = slice(h * HD + HD // 2, (h + 1) * HD)
                    t1 = tpool.tile([128, HD // 2], F32, tag="t1", bufs=4)
                    t2 = tpool.tile([128, HD // 2], F32, tag="t2", bufs=4)
                    nc.vector.tensor_mul(t1, ps_q[:, e], cq_sb[:, i, :])
                    nc.vector.tensor_mul(t2, ps_q[:, o], sq_sb[:, i, :])
                    nc.vector.tensor_sub(qrot_n[:, e], t1, t2)
                    t3 = tpool.tile([128, HD // 2], F32, tag="t1", bufs=4)
                    t4 = tpool.tile([128, HD // 2], F32, tag="t2", bufs=4)
                    nc.vector.tensor_mul(t3, ps_q[:, e], sq_sb[:, i, :])
                    nc.vector.tensor_mul(t4, ps_q[:, o], cq_sb[:, i, :])
                    nc.vector.tensor_add(qrot_n[:, o], t3, t4)
                krot_n = tpool.tile([128, HD], BF16, tag="krot", bufs=2)
                e = slice(0, HD // 2)
                o = slice(HD // 2, HD)
                t1 = tpool.tile([128, HD // 2], F32, tag="t1", bufs=4)
                t2 = tpool.tile([128, HD // 2], F32, tag="t2", bufs=4)
                nc.vector.tensor_mul(t1, ps_kv[:, e], ck_sb[:, i, :])
                nc.vector.tensor_mul(t2, ps_kv[:, o], sk_sb[:, i, :])
                nc.vector.tensor_sub(krot_n[:, e], t1, t2)
                t3 = tpool.tile([128, HD // 2], F32, tag="t1", bufs=4)
                t4 = tpool.tile([128, HD // 2], F32, tag="t2", bufs=4)
                nc.vector.tensor_mul(t3, ps_kv[:, e], sk_sb[:, i, :])
                nc.vector.tensor_mul(t4, ps_kv[:, o], ck_sb[:, i, :])
                nc.vector.tensor_add(krot_n[:, o], t3, t4)
                # transpose to [hd, s] layouts for the score matmuls
                for h in range(QH):
                    nc.sync.dma_start(
                        qrotT[:, h, i * SB:(i + 1) * SB],
                        qrot_n[:, h * HD:(h + 1) * HD], transpose=True)
                nc.sync.dma_start(
                    krotT[:, i * SB:(i + 1) * SB], krot_n, transpose=True)

        # --- phase 2: attention -----------------------------------------
        for h in range(QH):
            for i in range(NSB):
                chunks = [(j, k, m) for j, (k, m) in enumerate(chunk_kind[i])
                          if k != "skip"]
                p_t = ppool.tile([128, S], BF16, tag="p")
                lpart = tpool.tile([128, NTC], F32, tag="lpart", bufs=4)
                for ci, (j, kind, mi) in enumerate(chunks):
                    ps_s = psum.tile([128, TC], F32, tag="s")
                    nc.tensor.matmul(ps_s, qrotT[:, h, i * SB:(i + 1) * SB],
                                     krotT[:, j * TC:(j + 1) * TC],
                                     start=True, stop=True)
                    if kind == "masked":
                        mblk = tpool.tile([128, TC], F32, tag="mblk", bufs=2)
                        nc.sync.dma_start(mblk, maskb[mi])
                        nc.vector.tensor_add(ps_s, ps_s, mblk)
                    nc.scalar.activation(
                        p_t[:, j * TC:(j + 1) * TC], ps_s,
                        mybir.ActivationFunctionType.Exp,
                        accum_out=lpart[:, ci:ci + 1])
                lsum = tpool.tile([128, 1], F32, tag="lsum", bufs=4)
                nc.vector.reduce_sum(lsum, lpart[:, :len(chunks)],
                                     axis=mybir.AxisListType.X)
                rl = tpool.tile([128, 1], F32, tag="rl", bufs=4)
                nc.vector.reciprocal(rl, lsum)
                tbs = av_tblocks[i]
                pT = ppool.tile([128, NTB, SB], BF16, tag="pT")
                for tb in tbs:
                    nc.sync.dma_start(pT[:, tb, :],
                                      p_t[:, tb * 128:(tb + 1) * 128],
                                      transpose=True)
                ps_a = psum.tile([128, HD], F32, tag="a")
                for k, tb in enumerate(tbs):
                    nc.tensor.matmul(ps_a, pT[:, tb, :], v_sb[:, tb, :],
                                     start=(k == 0), stop=(k == len(tbs) - 1))
                attn_n = tpool.tile([128, HD], BF16, tag="attn_n", bufs=4)
                nc.scalar.activation(attn_n, ps_a,
                                     mybir.ActivationFunctionType.Copy,
                                     scale=rl)
                nc.sync.dma_start(attnT[:, h, i * SB:(i + 1) * SB], attn_n,
                                  transpose=True)

        # --- phase 3: output projection ---------------------------------
        for ob in range(D // 128):
            wo_ob = opool.tile([128, QH, 128], BF16, tag="wo_ob")
            nc.sync.dma_start(
                wo_ob, woT[:, ob * 128:(ob + 1) * 128].rearrange(
                    "(hb p) o -> p hb o", p=128))
            for sc in range(NSC):
                ps_o = psum.tile([128, SCHUNK], F32, tag="q")
                for hb in range(QH):
                    nc.tensor.matmul(
                        ps_o, wo_ob[:, hb, :],
                        attnT[:, hb, sc * SCHUNK:(sc + 1) * SCHUNK],
                        start=(hb == 0), stop=(hb == QH - 1))
                osb = opool.tile([128, SCHUNK], F32, tag="osb")
                nc.vector.tensor_copy(osb, ps_o)
                nc.sync.dma_start(
                    outT[ob * 128:(ob + 1) * 128,
                         sc * SCHUNK:(sc + 1) * SCHUNK], osb)
    nc.finalize()
    return nc


def kernel(x, wq, wk, wv, wo, cos, sin, cache, mask, start_pos):
    global _LAST_RESULTS
    x = np.asarray(x, np.float32)
    mask = np.asarray(mask, np.float32)
    cos = np.asarray(cos, np.float32)
    sin = np.asarray(sin, np.float32)

    chunk_kind, av_tblocks, mask_blocks = _classify_mask(mask)
    nc = _build_nc(chunk_kind, av_tblocks, mask_blocks.shape[0])

    # de-interleave permutation for the RoPE pair layout
    perm = np.concatenate([np.arange(0, HD, 2), np.arange(1, HD, 2)])
    xT = np.ascontiguousarray(x[0].T).astype(NPBF16)
    scale = np.float32(1.0 / np.sqrt(HD))

    in_maps = []
    for c in range(NCORES):
        wq_c = wq[c * QH * HD:(c + 1) * QH * HD].reshape(QH, HD, D)[:, perm]
        wq_c = wq_c.reshape(QH * HD, D)
        wk_c = wk[c * HD:(c + 1) * HD][perm]
        wv_c = wv[c * HD:(c + 1) * HD]
        wkv_c = np.concatenate([wk_c, wv_c], axis=0)      # [256, D]
        wo_c = wo[:, c * QH * HD:(c + 1) * QH * HD]        # [D, 512]
        in_maps.append({
            "xT": xT,
            "wqT": np.ascontiguousarray(wq_c.T).astype(NPBF16),
            "wkvT": np.ascontiguousarray(wkv_c.T).astype(NPBF16),
            "woT": np.ascontiguousarray(wo_c.T).astype(NPBF16),
            "cosq": np.ascontiguousarray(cos * scale),
            "sinq": np.ascontiguousarray(sin * scale),
            "cosk": np.ascontiguousarray(cos),
            "sink": np.ascontiguousarray(sin),
            "maskb": mask_blocks,
        })

    res = run_bass_kernel_spmd(nc, in_maps, core_ids=list(range(NCORES)),
                               trace=_TRACE)
    _LAST_RESULTS = res
    acc = np.zeros((D, S), np.float64)
    for r in res.results:
        acc += r["outT"].astype(np.float64)
    return acc.T.reshape(B, S, H * HD).astype(np.float32)


# revision 10
# speedup vs baseline: 2.4481x; 2.4481x over previous
"""TRN2 Bass kernel for nn_Attention_3728031613427.

GQA causal attention (B=1, S=2048, D=4096, H=32, KV=8, HD=128) with RoPE,
8-way tensor-parallel over KV heads. Each core computes 4 q-heads / 1 kv-head
and a full [D, S] partial of the output projection; partials are summed on
the host (the all-reduce of the sharding hint).

v2: scores are computed transposed (scoresT = kT.T @ qT) so the softmax
probabilities are born in the [t, s] layout AV needs -- no p transposes.
The softmax denominator l[s] falls out of the AV matmul via a ones-column
appended to V (column HD of the accumulation). All remaining 128x128
transposes (q/k/v/attn) run on the PE in transpose mode instead of the slow
DMA xbar.
"""

import sys

if "/opt/trn_rl_repo" not in sys.path:
    sys.path.insert(0, "/opt/trn_rl_repo")

from contextlib import ExitStack

import ml_dtypes
import numpy as np

import concourse.bass as bass
import concourse.mybir as mybir
import concourse.tile as tile
from concourse import bacc
from concourse.bass_utils import run_bass_kernel_spmd
from concourse.masks import make_identity

BF16 = mybir.dt.bfloat16
F32 = mybir.dt.float32
NPBF16 = ml_dtypes.bfloat16

B, S, D = 1, 2048, 4096
H, KV, HD = 32, 8, 128
NCORES = 8
QH = H // NCORES         # 4 q heads per core
SB = 128                 # s-block
TC = 512                 # s-chunk width for scoresT tiles
NSB = S // SB            # 16
NSC = S // TC            # 4
NTB = S // 128           # 16 t-blocks
DC = D // 128            # 32 contraction chunks
VW = HD + 1              # v with ones column -> l in column HD
NEG_THRESH = -1e8

_TRACE = False
_LAST_RESULTS = None


def _classify_mask(mask):
    """Classify [t-block 128, s-chunk 512] blocks of mask.T as skip / plain /
    masked, and per (s-block, t-block) AV inclusion."""
    tchunk_kind = []     # [sc][tb] -> (kind, mask_idx)
    mask_blocks = []
    for sc in range(NSC):
        row = []
        for tb in range(NTB):
            blk = mask[sc * TC:(sc + 1) * TC, tb * SB:(tb + 1) * SB]  # [s, t]
            if (blk <= NEG_THRESH).all():
                row.append(("skip", -1))
            elif (blk == 0.0).all():
                row.append(("plain", -1))
            else:
                row.append(("masked", len(mask_blocks)))
                mask_blocks.append(np.ascontiguousarray(blk.T))  # [t, s]
        tchunk_kind.append(row)
    av_tblocks = []
    for i in range(NSB):
        tbs = []
        for tb in range(NTB):
            sub = mask[i * SB:(i + 1) * SB, tb * SB:(tb + 1) * SB]
            if not (sub <= NEG_THRESH).all() and \
                    tchunk_kind[i // (TC // SB)][tb][0] != "skip":
                tbs.append(tb)
        av_tblocks.append(tbs)
    if not mask_blocks:
        mask_blocks.append(np.zeros((SB, TC), np.float32))
    return tchunk_kind, av_tblocks, np.stack(mask_blocks).astype(np.float32)


def _build_nc(tchunk_kind, av_tblocks, nmask):
    nc = bacc.Bacc()

    xT = nc.declare_dram_parameter("xT", [D, S], BF16, isOutput=False)
    wqT = nc.declare_dram_parameter("wqT", [D, QH * HD], BF16, isOutput=False)
    wkvT = nc.declare_dram_parameter("wkvT", [D, 2 * HD], BF16, isOutput=False)
    woT = nc.declare_dram_parameter("woT", [QH * HD, D], BF16, isOutput=False)
    # cos/sin tables: q tables replicated 4x across heads (scaled by
    # 1/sqrt(HD)), k tables single-head
    cq4 = nc.declare_dram_parameter("cq4", [S, QH * 64], BF16, isOutput=False)
    sq4 = nc.declare_dram_parameter("sq4", [S, QH * 64], BF16, isOutput=False)
    ck1 = nc.declare_dram_parameter("ck1", [S, 64], BF16, isOutput=False)
    sk1 = nc.declare_dram_parameter("sk1", [S, 64], BF16, isOutput=False)
    maskb = nc.declare_dram_parameter("maskb", [nmask, SB, TC], F32, isOutput=False)
    outT = nc.declare_dram_parameter("outT", [D, S], F32, isOutput=True)

    with tile.TileContext(nc) as tc, ExitStack() as ctx:
        wpool = ctx.enter_context(tc.tile_pool(name="wpool", bufs=1))
        xpool = ctx.enter_context(tc.tile_pool(name="xpool", bufs=2))
        rpool = ctx.enter_context(tc.tile_pool(name="rpool", bufs=1))
        tpool = ctx.enter_context(tc.tile_pool(name="tpool", bufs=4))
        ppool = ctx.enter_context(tc.tile_pool(name="ppool", bufs=2))
        opool = ctx.enter_context(tc.tile_pool(name="opool", bufs=3))
        psum = ctx.enter_context(tc.tile_pool(name="psum", bufs=4, space="PSUM"))

        ident = wpool.tile([128, 128], BF16)
        make_identity(nc, ident)

        # --- resident loads ---------------------------------------------
        wq_sb = wpool.tile([128, DC, QH * HD], BF16)
        nc.sync.dma_start(wq_sb, wqT.rearrange("(dc p) m -> p dc m", p=128))
        wkv_sb = wpool.tile([128, DC, 2 * HD], BF16)
        nc.scalar.dma_start(wkv_sb, wkvT.rearrange("(dc p) m -> p dc m", p=128))

        cq_sb = wpool.tile([128, NSB, QH * 64], BF16)
        nc.sync.dma_start(cq_sb, cq4.rearrange("(i p) f -> p i f", p=128))
        sq_sb = wpool.tile([128, NSB, QH * 64], BF16)
        nc.scalar.dma_start(sq_sb, sq4.rearrange("(i p) f -> p i f", p=128))
        ck_sb = wpool.tile([128, NSB, 64], BF16)
        nc.sync.dma_start(ck_sb, ck1.rearrange("(i p) f -> p i f", p=128))
        sk_sb = wpool.tile([128, NSB, 64], BF16)
        nc.scalar.dma_start(sk_sb, sk1.rearrange("(i p) f -> p i f", p=128))

        qrotT = rpool.tile([128, QH, S], BF16)   # [hd', h, s]
        krotT = rpool.tile([128, S], BF16)       # [hd', t]
        v_sb = rpool.tile([128, NTB, VW], BF16)  # [t-in-block, tb, d'+1]
        attnT = rpool.tile([128, QH, S], BF16)   # [d', h, s]

        def h3(ap, width):
            return ap.rearrange("p (h x) -> p h x", x=HD)[:, :, 0:width]

        def o3(ap, width):
            return ap.rearrange("p (h x) -> p h x", x=HD)[:, :, HD // 2:HD // 2 + width]

        # --- phase 1: projections + RoPE --------------------------------
        XCH = 256  # x streaming chunk (doesn't affect matmul shapes)
        for xi in range(S // XCH):
            xc = xpool.tile([128, DC, XCH], BF16, tag="xc")
            nc.sync.dma_start(
                xc, xT[:, xi * XCH:(xi + 1) * XCH].rearrange(
                    "(dc p) s -> p dc s", p=128))
            for ib in range(XCH // SB):
                i = xi * (XCH // SB) + ib
                sblk = slice(ib * SB, (ib + 1) * SB)
                ps_q = psum.tile([128, QH * HD], F32, tag="b2")
                ps_kv = psum.tile([128, 2 * HD], F32, tag="b1")
                for dc in range(DC):
                    nc.tensor.matmul(ps_q, xc[:, dc, sblk], wq_sb[:, dc, :],
                                     start=(dc == 0), stop=(dc == DC - 1))
                for dc in range(DC):
                    nc.tensor.matmul(ps_kv, xc[:, dc, sblk], wkv_sb[:, dc, :],
                                     start=(dc == 0), stop=(dc == DC - 1))
                # RoPE on q: all 4 heads in one 3D-AP op per step
                qrot_n = tpool.tile([128, QH * HD], BF16, tag="qrot", bufs=2)
                c4 = cq_sb[:, i].rearrange("p (h x) -> p h x", x=64)
                s4 = sq_sb[:, i].rearrange("p (h x) -> p h x", x=64)
                t1 = tpool.tile([128, QH, 64], F32, tag="t1")
                t2 = tpool.tile([128, QH, 64], F32, tag="t2")
                nc.vector.tensor_mul(t1, h3(ps_q, 64), c4)
                nc.vector.tensor_mul(t2, o3(ps_q, 64), s4)
                nc.vector.tensor_sub(h3(qrot_n, 64), t1, t2)
                t3 = tpool.tile([128, QH, 64], F32, tag="t1")
                t4 = tpool.tile([128, QH, 64], F32, tag="t2")
                nc.vector.tensor_mul(t3, h3(ps_q, 64), s4)
                nc.vector.tensor_mul(t4, o3(ps_q, 64), c4)
                nc.vector.tensor_add(o3(qrot_n, 64), t3, t4)
                # RoPE on k; v is already [t, d'] -- copy straight in
                kv_n = tpool.tile([128, HD], BF16, tag="kv_n", bufs=2)
                e = slice(0, 64)
                o = slice(64, HD)
                t5 = tpool.tile([128, 64], F32, tag="t5")
                t6 = tpool.tile([128, 64], F32, tag="t6")
                nc.vector.tensor_mul(t5, ps_kv[:, e], ck_sb[:, i])
                nc.vector.tensor_mul(t6, ps_kv[:, o], sk_sb[:, i])
                nc.vector.tensor_sub(kv_n[:, e], t5, t6)
                t7 = tpool.tile([128, 64], F32, tag="t5")
                t8 = tpool.tile([128, 64], F32, tag="t6")
                nc.vector.tensor_mul(t7, ps_kv[:, e], sk_sb[:, i])
                nc.vector.tensor_mul(t8, ps_kv[:, o], ck_sb[:, i])
                nc.vector.tensor_add(kv_n[:, o], t7, t8)
                nc.vector.tensor_copy(v_sb[:, i, 0:HD], ps_kv[:, HD:2 * HD])
                nc.vector.memset(v_sb[:, i, HD:VW], 1.0)
                # transposes on the PE (transpose mode)
                for h in range(QH):
                    ps_t = psum.tile([128, 128], BF16, tag="b1")
                    nc.tensor.transpose(ps_t, qrot_n[:, h * HD:(h + 1) * HD],
                                        ident)
                    nc.vector.tensor_copy(qrotT[:, h, i * SB:(i + 1) * SB],
                                          ps_t)
                ps_t = psum.tile([128, 128], BF16, tag="b1")
                nc.tensor.transpose(ps_t, kv_n, ident)
                nc.vector.tensor_copy(krotT[:, i * SB:(i + 1) * SB], ps_t)

        # --- phase 2: attention (scoresT orientation) -------------------
        for h in range(QH):
            for sc in range(NSC):
                pT = ppool.tile([128, NTB, TC], BF16, tag="pT")
                for tb in range(NTB):
                    kind, mi = tchunk_kind[sc][tb]
                    if kind == "skip":
                        continue
                    ps_s = psum.tile([128, TC], F32, tag="b2")
                    nc.tensor.matmul(ps_s, krotT[:, tb * SB:(tb + 1) * SB],
                                     qrotT[:, h, sc * TC:(sc + 1) * TC],
                                     start=True, stop=True)
                    if kind == "masked":
                        mblk = tpool.tile([128, TC], F32, tag="mblk", bufs=2)
                        eng = nc.sync if tb % 2 == 0 else nc.scalar
                        eng.dma_start(mblk, maskb[mi])
                        nc.vector.tensor_add(ps_s, ps_s, mblk)
                    nc.scalar.activation(pT[:, tb, :], ps_s,
                                         mybir.ActivationFunctionType.Exp)
                for ib in range(TC // SB):
                    i = sc * (TC // SB) + ib
                    tbs = av_tblocks[i]
                    ps_a = psum.tile([128, VW], F32, tag="b1")
                    for k, tb in enumerate(tbs):
                        nc.tensor.matmul(ps_a,
                                         pT[:, tb, ib * SB:(ib + 1) * SB],
                                         v_sb[:, tb, :],
                                         start=(k == 0),
                                         stop=(k == len(tbs) - 1))
                    rl = tpool.tile([128, 1], F32, tag="rl")
                    nc.vector.reciprocal(rl, ps_a[:, HD:VW])
                    attn_n = tpool.tile([128, HD], BF16, tag="attn_n", bufs=2)
                    nc.scalar.activation(attn_n, ps_a[:, 0:HD],
                                         mybir.ActivationFunctionType.Copy,
                                         scale=rl)
                    ps_t = psum.tile([128, 128], BF16, tag="b1")
                    nc.tensor.transpose(ps_t, attn_n, ident)
                    nc.vector.tensor_copy(attnT[:, h, i * SB:(i + 1) * SB],
                                          ps_t)

        # --- phase 3: output projection ---------------------------------
        for ob in range(D // 128):
            wo_ob = opool.tile([128, QH, 128], BF16, tag="wo_ob")
            eng = nc.sync if ob % 2 == 0 else nc.scalar
            eng.dma_start(
                wo_ob, woT[:, ob * 128:(ob + 1) * 128].rearrange(
                    "(hb p) o -> p hb o", p=128))
            for sc in range(NSC):
                ps_o = psum.tile([128, TC], F32, tag="b2")
                for hb in range(QH):
                    nc.tensor.matmul(
                        ps_o, wo_ob[:, hb, :],
                        attnT[:, hb, sc * TC:(sc + 1) * TC],
                        start=(hb == 0), stop=(hb == QH - 1))
                osb = opool.tile([128, TC], F32, tag="osb")
                nc.scalar.copy(osb, ps_o)
                eng2 = nc.sync if sc % 2 == 0 else nc.scalar
                eng2.dma_start(
                    outT[ob * 128:(ob + 1) * 128, sc * TC:(sc + 1) * TC], osb)
    nc.finalize()
    return nc


def kernel(x, wq, wk, wv, wo, cos, sin, cache, mask, start_pos):
    global _LAST_RESULTS
    x = np.asarray(x, np.float32)
    mask = np.asarray(mask, np.float32)
    cos = np.asarray(cos, np.float32)
    sin = np.asarray(sin, np.float32)

    tchunk_kind, av_tblocks, mask_blocks = _classify_mask(mask)
    nc = _build_nc(tchunk_kind, av_tblocks, mask_blocks.shape[0])

    # de-interleave permutation for the RoPE pair layout
    perm = np.concatenate([np.arange(0, HD, 2), np.arange(1, HD, 2)])
    xT = np.ascontiguousarray(x[0].T).astype(NPBF16)
    scale = np.float32(1.0 / np.sqrt(HD))
    cq = np.tile(cos * scale, (1, QH)).astype(NPBF16)   # [S, 256]
    sq = np.tile(sin * scale, (1, QH)).astype(NPBF16)
    ck = cos.astype(NPBF16)
    sk = sin.astype(NPBF16)

    in_maps = []
    for c in range(NCORES):
        wq_c = wq[c * QH * HD:(c + 1) * QH * HD].reshape(QH, HD, D)[:, perm]
        wq_c = wq_c.reshape(QH * HD, D)
        wk_c = wk[c * HD:(c + 1) * HD][perm]
        wv_c = wv[c * HD:(c + 1) * HD]
        wkv_c = np.concatenate([wk_c, wv_c], axis=0)      # [256, D]
        wo_c = wo[:, c * QH * HD:(c + 1) * QH * HD]        # [D, 512]
        in_maps.append({
            "xT": xT,
            "wqT": np.ascontiguousarray(wq_c.T).astype(NPBF16),
            "wkvT": np.ascontiguousarray(wkv_c.T).astype(NPBF16),
            "woT": np.ascontiguousarray(wo_c.T).astype(NPBF16),
            "cq4": cq, "sq4": sq, "ck1": ck, "sk1": sk,
            "maskb": mask_blocks,
        })

    res = run_bass_kernel_spmd(nc, in_maps, core_ids=list(range(NCORES)),
                               trace=_TRACE)
    _LAST_RESULTS = res
    acc = np.zeros((D, S), np.float64)
    for r in res.results:
        acc += r["outT"].astype(np.float64)
    return acc.T.reshape(B, S, H * HD).astype(np.float32)
